# revision 4
# baseline (speedup 1.0000x reference)
"""Chamfer distance kernel v3 for Trainium2 (8 NeuronCores, data-parallel).

v2 computed the distance matrix TWICE (once per direction). v3 computes it
once per batch: S = 2x.t - |x|^2 - |t|^2 = -d^2 via K=13 split-precision bf16
matmul (PE, 512 mm/core). ScalarE copies each PSUM half to SBUF as bf16;
VectorE then does BOTH directions from the bf16 copy:
  - direction A (per-query NN): fold-tree max + reduce -> racc slots
  - direction B (per-target NN): running elementwise max into colrun (2x mode)
Per-unit finalize: colrun -> fp32, gpsimd partition-max, sum over targets.
Engines overlap: PE ~325us, ACT ~240us, DVE ~310us per repeat.

Repeats run inside For_i so the marginal-time metric sees only real compute.
"""
import sys

sys.path.insert(0, "/opt/trn_rl_repo")

import numpy as np
import ml_dtypes

import concourse.bacc as bacc
import concourse.bass as bass
import concourse.bass_isa as bass_isa
import concourse.tile as tile
from concourse import mybir
from concourse.alu_op_type import AluOpType
from concourse.bass_utils import run_bass_kernel_spmd

F32 = mybir.dt.float32
BF16 = mybir.dt.bfloat16
X = mybir.AxisListType.X
A = AluOpType

B, N, D3 = 16, 4096, 3
NCORES = 8
BPC = B // NCORES           # batches (= units) per core
RB = N // 128               # 32 query row-blocks per unit
K = 13                      # augmented contraction dim


def _build_nc(repeat: int = 1):
    nc = bacc.Bacc("TRN2", target_bir_lowering=False, debug=False, num_devices=NCORES)
    L_d = nc.dram_tensor("L", [K, BPC * N], BF16, kind="ExternalInput").ap()
    T_d = nc.dram_tensor("T", [K, BPC * N], BF16, kind="ExternalInput").ap()
    out_d = nc.dram_tensor("out", [1, 1], F32, kind="ExternalOutput").ap()

    with tile.TileContext(nc) as tc:
        import contextlib
        with contextlib.ExitStack() as ctx:
            pool = ctx.enter_context(tc.tile_pool(name="p", bufs=1))
            psum = ctx.enter_context(tc.tile_pool(name="ps", bufs=1, space="PSUM"))
            L = pool.tile([K, BPC * N], BF16, name="L")
            T = pool.tile([K, BPC * N], BF16, name="T")
            racc = pool.tile([128, BPC * RB], F32, name="racc")
            dbufs = [pool.tile([128, N], BF16, name=f"db{i}") for i in range(2)]
            t2048 = pool.tile([128, 2048], BF16, name="t2048")
            t1024 = pool.tile([128, 1024], BF16, name="t1024")
            colr = [pool.tile([128, N], BF16, name=f"colr{u}") for u in range(BPC)]
            colf = pool.tile([128, N], F32, name="colf")
            colg = pool.tile([128, N], F32, name="colg")
            cs = [pool.tile([1, 1], F32, name=f"cs{u}") for u in range(BPC)]
            rs = pool.tile([128, 1], F32, name="rs")
            rsr = pool.tile([128, 1], F32, name="rsr")
            tot = pool.tile([1, 1], F32, name="tot")
            s_out = pool.tile([1, 1], F32, name="s_out")
            pA = psum.tile([128, 2048], F32, name="pA")
            pB = psum.tile([128, 2048], F32, name="pB")

            nc.gpsimd.dma_start(L[:], L_d[:])
            nc.gpsimd.dma_start(T[:], T_d[:])

            with tc.For_i(0, repeat) as rep:
                for u in range(BPC):
                    for rb in range(RB):
                        it = u * RB + rb
                        lhsT = L[:, it * 128 : (it + 1) * 128]
                        db = dbufs[rb % 2]
                        for h, ps in ((0, pA), (1, pB)):
                            for c in range(4):
                                off = u * N + h * 2048 + c * 512
                                nc.tensor.matmul(
                                    ps[:, c * 512 : (c + 1) * 512],
                                    lhsT,
                                    T[:, off : off + 512],
                                )
                            nc.scalar.copy(db[:, h * 2048 : (h + 1) * 2048], ps[:])
                        # direction A: fold-tree max over targets
                        nc.vector.tensor_tensor(t2048[:], db[:, 0:2048],
                                                db[:, 2048:4096], op=A.max)
                        nc.vector.tensor_tensor(t1024[:], t2048[:, 0:1024],
                                                t2048[:, 1024:2048], op=A.max)
                        nc.vector.tensor_reduce(racc[:, it : it + 1],
                                                t1024[:], axis=X, op=A.max)
                        # direction B: running columnwise max
                        if rb == 0:
                            nc.vector.tensor_copy(colr[u][:], db[:])
                        else:
                            nc.vector.tensor_tensor(colr[u][:], colr[u][:],
                                                    db[:], op=A.max)
                    # unit finalize: partition-max of colrun, sum over targets
                    nc.vector.tensor_copy(colf[:], colr[u][:])
                    nc.gpsimd.partition_all_reduce(colg[:], colf[:], channels=128,
                                                   reduce_op=bass_isa.ReduceOp.max)
                    nc.vector.reduce_sum(cs[u][:], colg[0:1, :], axis=X)
                # rowacc grand sum + partition sum
                nc.vector.reduce_sum(rs[:], racc[:], axis=X)
                nc.gpsimd.partition_all_reduce(rsr[:], rs[:], channels=128,
                                               reduce_op=bass_isa.ReduceOp.add)
                nc.vector.tensor_add(tot[:], cs[0][:], cs[1][:])
                nc.vector.tensor_add(tot[:], tot[:], rsr[0:1, 0:1])
                nc.scalar.mul(s_out[:], tot[:], -1.0 / N)
                nc.gpsimd.dma_start(out_d[:], s_out[:])
    nc.compile()
    return nc


def _split_hi_lo(a):
    hi = a.astype(ml_dtypes.bfloat16)
    lo = (a - hi.astype(np.float32)).astype(ml_dtypes.bfloat16)
    return hi, lo


def _aug(Q, Tg):
    """lhsT rows (queries) / rhs rows (targets), both [13, 4096] bf16.
    S = sum_k L[k,:,None]*T[k,None,:] = 2 q.t - |q|^2 - |t|^2 = -||q-t||^2
    """
    Qh, Ql = _split_hi_lo(Q)
    Th, Tl = _split_hi_lo(Tg)
    q2 = (Q.astype(np.float64) ** 2).sum(1).astype(np.float32)
    t2 = (Tg.astype(np.float64) ** 2).sum(1).astype(np.float32)
    q2h, q2l = _split_hi_lo(q2)
    t2h, t2l = _split_hi_lo(t2)
    n = Q.shape[0]
    Lr = np.zeros((K, n), ml_dtypes.bfloat16)
    Tr = np.zeros((K, n), ml_dtypes.bfloat16)
    for k in range(3):
        Lr[3 * k + 0] = (2.0 * Qh[:, k].astype(np.float32)).astype(ml_dtypes.bfloat16)
        Tr[3 * k + 0] = Th[:, k]
        Lr[3 * k + 1] = Lr[3 * k + 0]
        Tr[3 * k + 1] = Tl[:, k]
        Lr[3 * k + 2] = (2.0 * Ql[:, k].astype(np.float32)).astype(ml_dtypes.bfloat16)
        Tr[3 * k + 2] = Th[:, k]
    Lr[9] = -q2h; Tr[9] = 1.0
    Lr[10] = -q2l; Tr[10] = 1.0
    Lr[11] = -1.0; Tr[11] = t2h
    Lr[12] = -1.0; Tr[12] = t2l
    return Lr, Tr


def _build_operands(x, y):
    x = np.ascontiguousarray(x, np.float32)
    y = np.ascontiguousarray(y, np.float32)
    in_maps = []
    for core in range(NCORES):
        Ls, Ts = [], []
        for b in range(core * BPC, (core + 1) * BPC):
            Lr, Tr = _aug(x[b], y[b])
            Ls.append(Lr)
            Ts.append(Tr)
        in_maps.append({"L": np.ascontiguousarray(np.concatenate(Ls, axis=1)),
                        "T": np.ascontiguousarray(np.concatenate(Ts, axis=1))})
    return in_maps


_NC_CACHE = {}


def _get_nc(repeat: int = 1):
    if repeat not in _NC_CACHE:
        _NC_CACHE[repeat] = _build_nc(repeat)
    return _NC_CACHE[repeat]


def kernel(x, y):
    x = np.asarray(x, dtype=np.float32)
    y = np.asarray(y, dtype=np.float32)
    assert x.shape == (B, N, D3) and y.shape == (B, N, D3)
    in_maps = _build_operands(x, y)
    nc = _get_nc(1)
    res = run_bass_kernel_spmd(nc, in_maps, core_ids=list(range(NCORES)))
    total = sum(float(res.results[i]["out"][0, 0]) for i in range(NCORES))
    return np.float32(total / B)


# revision 5
# speedup vs baseline: 4.2525x; 4.2525x over previous
"""Chamfer distance kernel v3b for Trainium2 (8 NeuronCores, data-parallel).

v2 computed the distance matrix TWICE (once per direction). v3 computes it
once per batch: S = 2x.t - |x|^2 - |t|^2 = -d^2 via K=13 split-precision bf16
matmul (PE, 512 mm/core). ScalarE copies each PSUM half to SBUF as bf16;
VectorE then does BOTH directions from the bf16 copy:
  - direction A (per-query NN): fold-tree max + reduce -> racc slots
  - direction B (per-target NN): running elementwise max into colrun (2x mode)
Per-unit finalize: colrun -> fp32, gpsimd partition-max, sum over targets.
Engines overlap: PE ~325us, ACT ~240us, DVE ~310us per repeat.

Repeats run inside For_i so the marginal-time metric sees only real compute.
"""
import sys

sys.path.insert(0, "/opt/trn_rl_repo")

import numpy as np
import ml_dtypes

import concourse.bacc as bacc
import concourse.bass as bass
import concourse.bass_isa as bass_isa
import concourse.tile as tile
from concourse import mybir
from concourse.alu_op_type import AluOpType
from concourse.bass_utils import run_bass_kernel_spmd

F32 = mybir.dt.float32
BF16 = mybir.dt.bfloat16
X = mybir.AxisListType.X
A = AluOpType

B, N, D3 = 16, 4096, 3
NCORES = 8
BPC = B // NCORES           # batches (= units) per core
RB = N // 128               # 32 query row-blocks per unit
K = 13                      # augmented contraction dim


def _build_nc(repeat: int = 1):
    nc = bacc.Bacc("TRN2", target_bir_lowering=False, debug=False, num_devices=NCORES)
    L_d = nc.dram_tensor("L", [K, BPC * N], BF16, kind="ExternalInput").ap()
    T_d = nc.dram_tensor("T", [K, BPC * N], BF16, kind="ExternalInput").ap()
    out_d = nc.dram_tensor("out", [1, 1], F32, kind="ExternalOutput").ap()

    with tile.TileContext(nc) as tc:
        import contextlib
        with contextlib.ExitStack() as ctx:
            pool = ctx.enter_context(tc.tile_pool(name="p", bufs=1))
            psum = ctx.enter_context(tc.tile_pool(name="ps", bufs=1, space="PSUM"))
            L = pool.tile([K, BPC * N], BF16, name="L")
            T = pool.tile([K, BPC * N], BF16, name="T")
            racc = pool.tile([128, BPC * RB], F32, name="racc")
            dbufs = [pool.tile([128, N], BF16, name=f"db{i}") for i in range(2)]
            t2048 = pool.tile([128, 2048], BF16, name="t2048")
            t1024 = pool.tile([128, 1024], BF16, name="t1024")
            t512 = pool.tile([128, 512], BF16, name="t512")
            colr = [pool.tile([128, N], BF16, name=f"colr{u}") for u in range(BPC)]
            colgb = pool.tile([128, N], BF16, name="colgb")
            cs = [pool.tile([1, 1], F32, name=f"cs{u}") for u in range(BPC)]
            rs = pool.tile([128, 1], F32, name="rs")
            rsr = pool.tile([128, 1], F32, name="rsr")
            tot = pool.tile([1, 1], F32, name="tot")
            s_out = pool.tile([1, 1], F32, name="s_out")
            pA = psum.tile([128, 2048], F32, name="pA")
            pB = psum.tile([128, 2048], F32, name="pB")

            nc.gpsimd.dma_start(L[:], L_d[:])
            nc.gpsimd.dma_start(T[:], T_d[:])

            with tc.For_i(0, repeat) as rep:
                for u in range(BPC):
                    for rb in range(RB):
                        it = u * RB + rb
                        lhsT = L[:, it * 128 : (it + 1) * 128]
                        db = dbufs[rb % 2]
                        for h, ps in ((0, pA), (1, pB)):
                            for c in range(4):
                                off = u * N + h * 2048 + c * 512
                                nc.tensor.matmul(
                                    ps[:, c * 512 : (c + 1) * 512],
                                    lhsT,
                                    T[:, off : off + 512],
                                )
                            nc.scalar.copy(db[:, h * 2048 : (h + 1) * 2048], ps[:])
                        # direction A: fold-tree max over targets
                        nc.vector.tensor_tensor(t2048[:], db[:, 0:2048],
                                                db[:, 2048:4096], op=A.max)
                        nc.vector.tensor_tensor(t1024[:], t2048[:, 0:1024],
                                                t2048[:, 1024:2048], op=A.max)
                        nc.vector.tensor_tensor(t512[:], t1024[:, 0:512],
                                                t1024[:, 512:1024], op=A.max)
                        nc.vector.tensor_reduce(racc[:, it : it + 1],
                                                t512[:], axis=X, op=A.max)
                        # direction B: running columnwise max
                        if rb == 0:
                            nc.vector.tensor_copy(colr[u][:], db[:])
                        else:
                            nc.vector.tensor_tensor(colr[u][:], colr[u][:],
                                                    db[:], op=A.max)
                    # unit finalize: partition-max of colrun, sum over targets
                    nc.gpsimd.partition_all_reduce(colgb[:], colr[u][:], channels=128,
                                                   reduce_op=bass_isa.ReduceOp.max)
                    nc.vector.reduce_sum(cs[u][:], colgb[0:1, :], axis=X)
                # rowacc grand sum + partition sum
                nc.vector.reduce_sum(rs[:], racc[:], axis=X)
                nc.gpsimd.partition_all_reduce(rsr[:], rs[:], channels=128,
                                               reduce_op=bass_isa.ReduceOp.add)
                nc.vector.tensor_add(tot[:], cs[0][:], cs[1][:])
                nc.vector.tensor_add(tot[:], tot[:], rsr[0:1, 0:1])
                nc.scalar.mul(s_out[:], tot[:], -1.0 / N)
                nc.gpsimd.dma_start(out_d[:], s_out[:])
    nc.compile()
    return nc


def _split_hi_lo(a):
    hi = a.astype(ml_dtypes.bfloat16)
    lo = (a - hi.astype(np.float32)).astype(ml_dtypes.bfloat16)
    return hi, lo


def _aug(Q, Tg):
    """lhsT rows (queries) / rhs rows (targets), both [13, 4096] bf16.
    S = sum_k L[k,:,None]*T[k,None,:] = 2 q.t - |q|^2 - |t|^2 = -||q-t||^2
    """
    Qh, Ql = _split_hi_lo(Q)
    Th, Tl = _split_hi_lo(Tg)
    q2 = (Q.astype(np.float64) ** 2).sum(1).astype(np.float32)
    t2 = (Tg.astype(np.float64) ** 2).sum(1).astype(np.float32)
    q2h, q2l = _split_hi_lo(q2)
    t2h, t2l = _split_hi_lo(t2)
    n = Q.shape[0]
    Lr = np.zeros((K, n), ml_dtypes.bfloat16)
    Tr = np.zeros((K, n), ml_dtypes.bfloat16)
    for k in range(3):
        Lr[3 * k + 0] = (2.0 * Qh[:, k].astype(np.float32)).astype(ml_dtypes.bfloat16)
        Tr[3 * k + 0] = Th[:, k]
        Lr[3 * k + 1] = Lr[3 * k + 0]
        Tr[3 * k + 1] = Tl[:, k]
        Lr[3 * k + 2] = (2.0 * Ql[:, k].astype(np.float32)).astype(ml_dtypes.bfloat16)
        Tr[3 * k + 2] = Th[:, k]
    Lr[9] = -q2h; Tr[9] = 1.0
    Lr[10] = -q2l; Tr[10] = 1.0
    Lr[11] = -1.0; Tr[11] = t2h
    Lr[12] = -1.0; Tr[12] = t2l
    return Lr, Tr


def _build_operands(x, y):
    x = np.ascontiguousarray(x, np.float32)
    y = np.ascontiguousarray(y, np.float32)
    in_maps = []
    for core in range(NCORES):
        Ls, Ts = [], []
        for b in range(core * BPC, (core + 1) * BPC):
            Lr, Tr = _aug(x[b], y[b])
            Ls.append(Lr)
            Ts.append(Tr)
        in_maps.append({"L": np.ascontiguousarray(np.concatenate(Ls, axis=1)),
                        "T": np.ascontiguousarray(np.concatenate(Ts, axis=1))})
    return in_maps


_NC_CACHE = {}


def _get_nc(repeat: int = 1):
    if repeat not in _NC_CACHE:
        _NC_CACHE[repeat] = _build_nc(repeat)
    return _NC_CACHE[repeat]


def kernel(x, y):
    x = np.asarray(x, dtype=np.float32)
    y = np.asarray(y, dtype=np.float32)
    assert x.shape == (B, N, D3) and y.shape == (B, N, D3)
    in_maps = _build_operands(x, y)
    nc = _get_nc(1)
    res = run_bass_kernel_spmd(nc, in_maps, core_ids=list(range(NCORES)))
    total = sum(float(res.results[i]["out"][0, 0]) for i in range(NCORES))
    return np.float32(total / B)
